# revision 41
# baseline (speedup 1.0000x reference)
"""Trainium2 Bass kernel for a 6-layer post-LN transformer encoder.

Model (per reference):
  h = (x @ Wemb + bemb) * sqrt(D) + posenc
  for l in 6:  h = LN(h + MHA_l(h))   (8 heads, dh=64, softmax over keys)

Sharding: pure data-parallel over batch. B=16 across 8 NeuronCores,
2 batch elements per core, weights replicated, no collectives.

Per-core structure:
  - h kept in BOTH layouts per element: S-major h_s (residual+LN) and
    D-major hT (matmul operand, float32r), refreshed each layer via PE
    transposes.
  - Q,K,V per element in bf16. V is stored head-padded (65 cols/head) with a
    ones column so the attention matmul Vp'.T @ exp(scoresT) yields the
    numerator rows (0..63) and softmax denominator (row 64) in one pass.
  - scoresT = K_h.T-major bf16 matmul (N=1024); exp fused with mask bias +
    1/8 scale on ScalarE, output bf16; denominator reciprocal broadcast
    across partitions with gpsimd.partition_broadcast.
  - QKV projections f32r (TF32-like fp32 mode), O-projection bf16 (attnT,
    Wo bf16); psum, LN and softmax statistics in fp32.
  - The two batch elements are software-pipelined half a layer apart, with
    instruction emission interleaved so each engine's (in-order) stream
    alternates between element-0 and element-1 work: PE-heavy QKV/O overlaps
    ScalarE-heavy attention.
"""
import numpy as np
import ml_dtypes

# -- model constants (hardcoded per contract) --
B, S, F, D, H, L = 16, 1024, 64, 512, 8, 6
DH = D // H          # 64
P = 128              # partitions
NS = S // P          # 8 s-chunks of 128
KC = D // P          # 4 d-chunks of 128
NH = 2               # s-halves of 512 (fp32 moving-operand limit)
NHW = S // NH        # 512
NCORES = 8
BLOC = B // NCORES   # 2
EPS = 1e-6
SQRT_D = float(np.sqrt(np.float32(D)))
SCALE = 1.0 / float(np.sqrt(np.float32(DH)))

_CACHE = {}


def _posenc_np():
    pos = np.arange(S)[:, None].astype(np.float32)
    i = np.arange(D)[None, :].astype(np.float32)
    angle = pos / np.power(10000.0, 2.0 * (i // 2) / np.float32(D)).astype(np.float32)
    angle[:, 0::2] = np.sin(angle[:, 0::2])
    angle[:, 1::2] = np.cos(angle[:, 1::2])
    return angle.astype(np.float32)  # [S, D]


def _build_nc():
    import concourse.bacc as bacc
    import concourse.mybir as mybir
    import concourse.tile as tile
    from concourse.masks import make_identity

    f32 = mybir.dt.float32
    f32r = mybir.dt.float32r
    bf16 = mybir.dt.bfloat16
    AF = mybir.ActivationFunctionType
    OP = mybir.AluOpType

    nc = bacc.Bacc("TRN2", target_bir_lowering=False, debug=False)

    # ---- DRAM io ----
    x2 = nc.dram_tensor("x2", [BLOC, S, F], f32, kind="ExternalInput")
    maskT = nc.dram_tensor("maskT", [BLOC, NS, P], f32, kind="ExternalInput")
    wq_d = nc.dram_tensor("wq", [L, D, D], f32r, kind="ExternalInput")
    wk_d = nc.dram_tensor("wk", [L, D, D], f32r, kind="ExternalInput")
    wv_d = nc.dram_tensor("wv", [L, D, D], f32r, kind="ExternalInput")
    wo_d = nc.dram_tensor("wo", [L, D, D], bf16, kind="ExternalInput")
    bq_d = nc.dram_tensor("bq", [L, D], f32, kind="ExternalInput")
    bk_d = nc.dram_tensor("bk", [L, D], f32, kind="ExternalInput")
    bv_d = nc.dram_tensor("bv", [L, D], f32, kind="ExternalInput")
    bo_d = nc.dram_tensor("bo", [L, D], f32, kind="ExternalInput")
    gm_d = nc.dram_tensor("gamma", [L, D], f32, kind="ExternalInput")
    bt_d = nc.dram_tensor("beta", [L, D], f32, kind="ExternalInput")
    we_d = nc.dram_tensor("wemb", [F, D], f32r, kind="ExternalInput")
    pe_d = nc.dram_tensor("pose", [S, D], f32, kind="ExternalInput")
    out2 = nc.dram_tensor("out2", [BLOC, S, D], f32, kind="ExternalOutput")

    with tile.TileContext(nc) as tc:
        persist = tc.alloc_tile_pool(name="persist", bufs=1)
        wpool = tc.alloc_tile_pool(name="wpool", bufs=1)
        stage = tc.alloc_tile_pool(name="stage", bufs=3)
        tmp = tc.alloc_tile_pool(name="tmp", bufs=3)
        expp = tc.alloc_tile_pool(name="expp", bufs=4)
        pmm = tc.alloc_tile_pool(name="pmm", bufs=2, space="PSUM")
        psc = tc.alloc_tile_pool(name="psc", bufs=2, space="PSUM")
        pat = tc.alloc_tile_pool(name="pat", bufs=2, space="PSUM")

        # ---- persistent state ----
        ident = persist.tile([P, P], f32, name="ident")
        make_identity(nc, ident[:])
        eps_t = persist.tile([P, 1], f32, name="eps_t")
        nc.vector.memset(eps_t[:], EPS)
        wemb_r = persist.tile([F, D], f32r, name="wemb_r")
        xT = persist.tile([F, S], f32r, name="xT")
        QT = [persist.tile([P, KC, S], bf16, name=f"QT{b}") for b in range(BLOC)]
        KT = [persist.tile([P, KC, S], bf16, name=f"KT{b}") for b in range(BLOC)]
        Vp = [persist.tile([P, NS, H * (DH + 1)], bf16, name=f"Vp{b}")
              for b in range(BLOC)]
        aT = [persist.tile([P, KC, S], bf16, name=f"aT{b}") for b in range(BLOC)]
        h_s = [persist.tile([P, NS, D], f32, name=f"h_s{b}") for b in range(BLOC)]
        hT = [persist.tile([P, KC, S], f32r, name=f"hT{b}") for b in range(BLOC)]
        mask_t = [persist.tile([P, NS], f32, name=f"mask_t{b}") for b in range(BLOC)]

        # ones columns of Vp (written once; per-layer V writes skip them)
        ones_t = persist.tile([P, NS * H], f32, name="ones_t")
        nc.vector.memset(ones_t[:], 1.0)
        for b in range(BLOC):
            nc.vector.tensor_copy(
                Vp[b][:].rearrange("p t (h j) -> p t h j", j=DH + 1)[:, :, :, DH:DH + 1],
                ones_t[:].rearrange("p (t h) -> p t h", h=H)[:, :, :, None])

        # ---- per-layer weight tiles ----
        w_q = wpool.tile([P, KC, D], f32r, name="w_q")
        w_k = wpool.tile([P, KC, D], f32r, name="w_k")
        w_v = wpool.tile([P, KC, D], f32r, name="w_v")
        w_o = wpool.tile([P, KC, D], bf16, name="w_o")
        bq_t = wpool.tile([P, KC], f32, name="bq_t")
        bk_t = wpool.tile([P, KC], f32, name="bk_t")
        bv_bc = wpool.tile([P, D], f32, name="bv_bc")
        bo_bc = wpool.tile([P, D], f32, name="bo_bc")
        gm_bc = wpool.tile([P, D], f32, name="gm_bc")
        bt_bc = wpool.tile([P, D], f32, name="bt_bc")

        # ================= emitters =================

        def emit_weight_dmas_qkv(l):
            for wd, wt in ((wq_d, w_q), (wk_d, w_k), (wv_d, w_v)):
                nc.sync.dma_start(
                    wt[:], wd[l].rearrange("(kc p) n -> p kc n", p=P))
            nc.sync.dma_start(bq_t[:], bq_d[l].rearrange("(c p) -> p c", p=P))
            nc.sync.dma_start(bk_t[:], bk_d[l].rearrange("(c p) -> p c", p=P))
            nc.sync.dma_start(bv_bc[:], bv_d[l][None, :].to_broadcast((P, D)))

        def emit_weight_dmas_o(l):
            nc.sync.dma_start(w_o[:], wo_d[l].rearrange("(kc p) n -> p kc n", p=P))
            nc.sync.dma_start(bo_bc[:], bo_d[l][None, :].to_broadcast((P, D)))
            nc.sync.dma_start(gm_bc[:], gm_d[l][None, :].to_broadcast((P, D)))
            nc.sync.dma_start(bt_bc[:], bt_d[l][None, :].to_broadcast((P, D)))

        def emit_qkv(b, g):
            """QKV group g in 0..23: 0-7 Q (dc,sh), 8-15 K, 16-23 V (tc)."""
            if g < 16:
                wt, bias_t, OT = (w_q, bq_t, QT[b]) if g < 8 else (w_k, bk_t, KT[b])
                gg = g % 8
                dc, sh = gg // NH, gg % NH
                pq = pmm.tile([P, NHW], f32, name="pq", tag="mm")
                for kc in range(KC):
                    nc.tensor.matmul(
                        pq[:],
                        wt[:, kc, dc * P:(dc + 1) * P],
                        hT[b][:, kc, sh * NHW:(sh + 1) * NHW],
                        start=(kc == 0), stop=(kc == KC - 1))
                nc.vector.tensor_scalar_add(
                    OT[:, dc, sh * NHW:(sh + 1) * NHW], pq[:], bias_t[:, dc:dc + 1])
            else:
                tcix = g - 16
                pv = pmm.tile([P, D], f32, name="pv", tag="mm")
                for kc in range(KC):
                    nc.tensor.matmul(
                        pv[:],
                        hT[b][:, kc, tcix * P:(tcix + 1) * P],
                        w_v[:, kc, :],
                        start=(kc == 0), stop=(kc == KC - 1))
                nc.vector.scalar_tensor_tensor(
                    out=Vp[b][:, tcix, :]
                    .rearrange("p (h j) -> p h j", j=DH + 1)[:, :, 0:DH],
                    in0=pv[:].rearrange("p (h j) -> p h j", j=DH),
                    scalar=1.0,
                    in1=bv_bc[:].rearrange("p (h j) -> p h j", j=DH),
                    op0=OP.mult, op1=OP.add)

        def emit_head(b, h):
            """Attention head h for element b -> writes aT[b] slice."""
            kcq = h // 2
            po = (h % 2) * DH
            c0 = h * (DH + 1)
            pa2 = [pat.tile([DH + 1, NHW], f32, name=f"pa{sh}", tag="at")
                   for sh in range(NH)]
            for tcix in range(NS):
                ps_t = psc.tile([P, S], f32, name="ps_t", tag="sc")
                for sh in range(NH):
                    nc.tensor.matmul(
                        ps_t[:, sh * NHW:(sh + 1) * NHW],
                        KT[b][po:po + DH, kcq, tcix * P:(tcix + 1) * P],
                        QT[b][po:po + DH, kcq, sh * NHW:(sh + 1) * NHW],
                        start=True, stop=True)
                e_t = expp.tile([P, S], bf16, name="e_t", tag="e_t")
                nc.scalar.activation(
                    out=e_t[:], in_=ps_t[:], func=AF.Exp,
                    bias=mask_t[b][:, tcix:tcix + 1], scale=SCALE)
                for sh in range(NH):
                    nc.tensor.matmul(
                        pa2[sh][:], Vp[b][:, tcix, c0:c0 + DH + 1],
                        e_t[:, sh * NHW:(sh + 1) * NHW],
                        start=(tcix == 0), stop=(tcix == NS - 1))
            for sh in range(NH):
                recip = tmp.tile([1, NHW], f32, name="recip", tag="recip", bufs=2)
                nc.vector.reciprocal(recip[:], pa2[sh][DH:DH + 1, :])
                rec_bc = tmp.tile([DH, NHW], f32, name="rec_bc", tag="rec_bc", bufs=2)
                nc.gpsimd.partition_broadcast(rec_bc[:], recip[:], channels=DH)
                nc.vector.tensor_mul(
                    aT[b][po:po + DH, kcq, sh * NHW:(sh + 1) * NHW],
                    pa2[sh][0:DH, :], rec_bc[:])

        def emit_transp(b, sc):
            for kc in range(KC):
                pt = pmm.tile([P, P], f32, name="pt", tag="mm")
                nc.tensor.transpose(
                    pt[:], h_s[b][:, sc, kc * P:(kc + 1) * P], ident[:])
                nc.vector.tensor_copy(hT[b][:, kc, sc * P:(sc + 1) * P], pt[:])

        def emit_oln(b, sc, last_layer):
            """O-projection + residual + LN for s-chunk sc."""
            po_t = pmm.tile([P, D], f32, name="po_t", tag="mm")
            for kc in range(KC):
                nc.tensor.matmul(
                    po_t[:],
                    aT[b][:, kc, sc * P:(sc + 1) * P],
                    w_o[:, kc, :],
                    start=(kc == 0), stop=(kc == KC - 1))
            resid = tmp.tile([P, D], f32, name="resid", tag="resid", bufs=3)
            nc.vector.tensor_add(resid[:], po_t[:], h_s[b][:, sc, :])
            nc.vector.tensor_add(resid[:], resid[:], bo_bc[:])
            stats = tmp.tile([P, 6], f32, name="stats", tag="stats")
            nc.vector.bn_stats(out=stats[:], in_=resid[:])
            mv = tmp.tile([P, 2], f32, name="mv", tag="mv")
            nc.vector.bn_aggr(out=mv[:], in_=stats[:])
            lnv = tmp.tile([P, 1], f32, name="lnv", tag="lnv")
            nc.scalar.activation(out=lnv[:], in_=mv[:, 1:2], func=AF.Ln, bias=eps_t[:])
            rstd = tmp.tile([P, 1], f32, name="rstd", tag="rstd")
            nc.scalar.activation(out=rstd[:], in_=lnv[:], func=AF.Exp, scale=-0.5)
            xc = tmp.tile([P, D], f32, name="xc", tag="xc", bufs=3)
            nc.vector.tensor_scalar_sub(xc[:], resid[:], mv[:, 0:1])
            nc.vector.scalar_tensor_tensor(
                out=xc[:], in0=xc[:], scalar=rstd[:],
                in1=gm_bc[:], op0=OP.mult, op1=OP.mult)
            nc.vector.tensor_add(h_s[b][:, sc, :], xc[:], bt_bc[:])

        def emit_embed(b):
            nc.sync.dma_start(mask_t[b][:], maskT[b].rearrange("c p -> p c"))
            for sc in range(NS):
                x_sb = stage.tile([P, F], f32, name="x_sb", tag="x_sb")
                nc.sync.dma_start(x_sb[:], x2[b, sc * P:(sc + 1) * P, :])
                pxt = pmm.tile([F, P], f32, name="pxt", tag="mm")
                nc.tensor.transpose(pxt[:], x_sb[:], ident[:])
                nc.vector.tensor_copy(xT[:, sc * P:(sc + 1) * P], pxt[:])
            for sc in range(NS):
                pe_t = stage.tile([P, D], f32, name="pe_t", tag="stage")
                nc.sync.dma_start(pe_t[:], pe_d[sc * P:(sc + 1) * P, :])
                pemb = pmm.tile([P, D], f32, name="pemb", tag="mm")
                nc.tensor.matmul(
                    pemb[:], xT[:, sc * P:(sc + 1) * P], wemb_r[:],
                    start=True, stop=True)
                nc.vector.scalar_tensor_tensor(
                    out=h_s[b][:, sc, :], in0=pemb[:], scalar=SQRT_D,
                    in1=pe_t[:], op0=OP.mult, op1=OP.add)
            for kc in range(KC):
                for sc in range(NS):
                    pt = pmm.tile([P, P], f32, name="pt", tag="mm")
                    nc.tensor.transpose(
                        pt[:], h_s[b][:, sc, kc * P:(kc + 1) * P], ident[:])
                    nc.vector.tensor_copy(hT[b][:, kc, sc * P:(sc + 1) * P], pt[:])

        # ================= schedule =================
        # Software pipeline: element 1 runs half a layer behind element 0.
        #   loop1(l): heads(b0) interleaved with QKV(b1)
        #   loop2(l): heads(b1) interleaved with O/LN/T(b0)
        #   loop3(l): O/LN/T(b1) interleaved with QKV(b0) of layer l+1
        nc.sync.dma_start(wemb_r[:], we_d[:, :])
        emit_embed(0)
        emit_embed(1)
        emit_weight_dmas_qkv(0)
        emit_weight_dmas_o(0)
        for g in range(3 * H):
            emit_qkv(0, g)
        for l in range(L):
            last = l == L - 1
            for h in range(H):
                emit_head(0, h)
                for j in range(3):
                    emit_qkv(1, 3 * h + j)
            for h in range(H):
                emit_head(1, h)
                emit_oln(0, h, last)
                if not last and h >= 4:
                    emit_transp(0, h - 4)
            if not last:
                emit_weight_dmas_qkv(l + 1)
                for sc in range(NS - 4, NS):
                    emit_transp(0, sc)
            for sc in range(NS):
                emit_oln(1, sc, last)
                if not last:
                    if sc >= 4:
                        emit_transp(1, sc - 4)
                    for j in range(3):
                        emit_qkv(0, 3 * sc + j)
            if not last:
                for sc in range(NS - 4, NS):
                    emit_transp(1, sc)
                emit_weight_dmas_o(l + 1)

        # ---- store ----
        for b in range(BLOC):
            nc.sync.dma_start(
                out2[b].rearrange("(c p) d -> p c d", p=P), h_s[b][:])

        pat.release()
        psc.release()
        pmm.release()
        expp.release()
        tmp.release()
        stage.release()
        wpool.release()
        persist.release()

    nc.compile()
    return nc


def _get_nc():
    if "nc" not in _CACHE:
        _CACHE["nc"] = _build_nc()
    return _CACHE["nc"]


def kernel(x, padding_mask, training, Wemb, bemb, Wq, bq, Wk, bk, Wv, bv,
           Wo, bo, gamma, beta):
    from concourse.bass_utils import run_bass_kernel_spmd

    nc = _get_nc()

    x = np.asarray(x, dtype=np.float32)
    padding_mask = np.asarray(padding_mask, dtype=np.float32)
    pose = _posenc_np() + np.asarray(bemb, np.float32)[None, :] * np.float32(SQRT_D)

    common = {
        "wq": np.ascontiguousarray(Wq, np.float32),
        "wk": np.ascontiguousarray(Wk, np.float32),
        "wv": np.ascontiguousarray(Wv, np.float32),
        "wo": np.ascontiguousarray(np.asarray(Wo, np.float32).astype(ml_dtypes.bfloat16)),
        "bq": np.ascontiguousarray(bq, np.float32),
        "bk": np.ascontiguousarray(bk, np.float32),
        "bv": np.ascontiguousarray(bv, np.float32),
        "bo": np.ascontiguousarray(bo, np.float32),
        "gamma": np.ascontiguousarray(gamma, np.float32),
        "beta": np.ascontiguousarray(beta, np.float32),
        "wemb": np.ascontiguousarray(Wemb, np.float32),
        "pose": np.ascontiguousarray(pose, np.float32),
    }
    in_maps = []
    for c in range(NCORES):
        xs = x[c * BLOC:(c + 1) * BLOC]
        m = padding_mask[c * BLOC:(c + 1) * BLOC, 0, 0, :] * np.float32(-1e9)
        in_maps.append({
            "x2": np.ascontiguousarray(xs),
            "maskT": np.ascontiguousarray(m.reshape(BLOC, NS, P)),
            **common,
        })

    res = run_bass_kernel_spmd(nc, in_maps, core_ids=list(range(NCORES)))
    out = np.concatenate([r["out2"] for r in res.results], axis=0)
    return out

